# revision 1
# baseline (speedup 1.0000x reference)
"""AdaConvBlock Trainium2 kernel: 8-core data-parallel (2 batch elems/core).

Per core (b=2, C=384, L=4096):
  LN1 -> adaLN modulate -> SLConv (1024-tap depthwise conv via four-step
  matmul FFT, N=4608=128x36, batch pair packed as complex) + D-skip ->
  gated residual -> LN2 -> modulate -> pointwise MLP (gelu) -> gated residual.

FFT: Cooley-Tukey twiddles folded into 36 per-n2 stationary matrices ->
pure matmul FFT, no twiddle pointwise work. PSUM is managed in explicit
2KB-bank slots (one matmul output per slot, has_written gives per-slot
overwrite/accumulate semantics).
"""
import os
import sys

sys.path.insert(0, "/opt/trn_rl_repo")

import numpy as np
import ml_dtypes

import concourse.bass as bass
import concourse.bacc as bacc
import concourse.tile as tile
from concourse import mybir
from concourse.bass_utils import run_bass_kernel_spmd

F32 = mybir.dt.float32
BF16 = mybir.dt.bfloat16
AX = mybir.AluOpType
AF = mybir.ActivationFunctionType

B, C, L = 16, 384, 4096
NCORES = 8
BPC = B // NCORES
CT = 3
NCH = 8
LCH = 512
N, N1, N2 = 4608, 128, 36
KLEN = 1024
SS = 512
NG = 8
GS = 48
NSUB = 16
EPS = 1e-5
DECAY = 2.0
NS, KS = 6, 32

_last_results = None


def _make_consts():
    k1 = np.arange(N1)
    n1 = np.arange(N1)
    W128 = np.exp(-2j * np.pi * np.outer(k1, n1) / N1)
    W36 = np.exp(-2j * np.pi * np.outer(np.arange(N2), np.arange(N2)) / N2)
    fA = np.zeros((N2, N1, N1), complex)
    iA = np.zeros((N2, N1, N1), complex)
    for n2 in range(N2):
        M = np.exp(-2j * np.pi * n2 * k1 / N)[:, None] * W128      # [k1,n1]
        fA[n2] = M.T                                               # lhsT [n1,k1]
        IA = (np.exp(2j * np.pi * n2 * k1 / N)[None, :] * np.conj(W128).T) / N
        iA[n2] = IA.T                                              # lhsT [k1,n1]

    def blockdiag3(Mx):
        out = np.zeros((108, 108), complex)
        for s in range(3):
            out[s * 36:(s + 1) * 36, s * 36:(s + 1) * 36] = Mx
        return out

    fB = blockdiag3(W36.T)
    iB = blockdiag3(np.conj(W36).T)

    def bf(x):
        return np.ascontiguousarray(x).astype(ml_dtypes.bfloat16)

    return {
        "fA_re": bf(fA.real), "fA_im": bf(fA.imag), "fA_imn": bf(-fA.imag),
        "iA_re": bf(iA.real), "iA_im": bf(iA.imag), "iA_imn": bf(-iA.imag),
        "fB_re": bf(fB.real), "fB_im": bf(fB.imag), "fB_imn": bf(-fB.imag),
        "iB_re": bf(iB.real), "iB_im": bf(iB.imag), "iB_imn": bf(-iB.imag),
        "ident": bf(np.eye(128)),
        "ones_bf": bf(np.ones((128, 128))),
    }


# bank-slot offset for per-n2 FFT matmul outputs: 10 slots of 48 per 2KB bank
def _n2off(n2):
    return (n2 // 10) * 512 + (n2 % 10) * 48


def build_graph():
    nc = bacc.Bacc(None)

    x_e = nc.declare_dram_parameter("x", [BPC, C, L], F32, isOutput=False)
    tc_e = nc.declare_dram_parameter("t_cond", [BPC, C // 3, L], F32, isOutput=False)
    ker_e = nc.declare_dram_parameter("kernels", [NS, 1, C, KS], F32, isOutput=False)
    d_e = nc.declare_dram_parameter("DT", [128, CT], F32, isOutput=False)
    adawT_e = nc.declare_dram_parameter("ada_wT", [C // 3, 6 * C], BF16, isOutput=False)
    adab_e = nc.declare_dram_parameter("ada_bT", [128, 18], F32, isOutput=False)
    w1T_e = nc.declare_dram_parameter("w1T", [C, C], BF16, isOutput=False)
    b1_e = nc.declare_dram_parameter("b1T", [128, CT], F32, isOutput=False)
    w2T_e = nc.declare_dram_parameter("w2T", [C, C], BF16, isOutput=False)
    b2_e = nc.declare_dram_parameter("b2T", [128, CT], F32, isOutput=False)
    cshapes = {
        "fA_re": [N2, N1, N1], "fA_im": [N2, N1, N1], "fA_imn": [N2, N1, N1],
        "iA_re": [N2, N1, N1], "iA_im": [N2, N1, N1], "iA_imn": [N2, N1, N1],
        "fB_re": [108, 108], "fB_im": [108, 108], "fB_imn": [108, 108],
        "iB_re": [108, 108], "iB_im": [108, 108], "iB_imn": [108, 108],
        "ident": [128, 128], "ones_bf": [128, 128],
    }
    cst = {nm: nc.declare_dram_parameter(nm, shp, BF16, isOutput=False)
           for nm, shp in cshapes.items()}
    out_e = nc.declare_dram_parameter("out", [BPC, C, L], F32, isOutput=True)

    ymod_d = nc.dram_tensor("ymod", [BPC, C, N], BF16)
    yconv_d = nc.dram_tensor("yconv", [BPC, C, N], BF16)
    kpad_d = nc.dram_tensor("kpad", [C, 29 * N2], BF16)

    MM = nc.tensor.matmul

    with tile.TileContext(nc) as tc, \
         nc.allow_low_precision(reason="bf16 datapath, fp32 psum accumulation"), \
         tc.tile_pool(name="sing", bufs=1) as sing:
        if True:
            zc = sing.tile([128, 1], F32)
            nc.vector.memset(zc, 0.0)
            nc.const_aps.aps[(F32, 0.0)] = zc[:, :]
            ec = sing.tile([128, 1], F32)
            nc.vector.memset(ec, EPS)
            nc.const_aps.aps[(F32, EPS)] = ec[:, :]
            sb = {}
            for qi, nm in enumerate(("fA_re", "fA_im", "fA_imn", "iA_re",
                                     "iA_im", "iA_imn")):
                t = sing.tile([N1, N2, N1], BF16, tag=nm)
                deng = nc.sync if qi % 2 == 0 else nc.gpsimd
                deng.dma_start(out=t, in_=cst[nm].rearrange("a b c -> b a c"))
                sb[nm] = t
            for nm in ("fB_re", "fB_im", "fB_imn", "iB_re", "iB_im", "iB_imn"):
                t = sing.tile([108, 108], BF16, tag=nm)
                nc.sync.dma_start(out=t, in_=cst[nm][:, :])
                sb[nm] = t
            ident = sing.tile([128, 128], BF16)
            nc.sync.dma_start(out=ident, in_=cst["ident"][:, :])
            ident32 = sing.tile([128, 128], F32)
            nc.scalar.activation(ident32, ident, AF.Copy)
            ones_bf = sing.tile([128, 128], BF16)
            nc.sync.dma_start(out=ones_bf, in_=cst["ones_bf"][:, :])
            adawT = sing.tile([128, 18, 128], BF16)
            nc.sync.dma_start(out=adawT,
                              in_=adawT_e.rearrange("k (c o) -> k c o", c=18))
            adab = sing.tile([128, 18], F32)
            nc.sync.dma_start(out=adab, in_=adab_e[:, :])
            w1T = sing.tile([128, CT, C], BF16)
            nc.sync.dma_start(out=w1T,
                              in_=w1T_e.rearrange("(a k) o -> k a o", k=128))
            w2T = sing.tile([128, CT, C], BF16)
            nc.sync.dma_start(out=w2T,
                              in_=w2T_e.rearrange("(a k) o -> k a o", k=128))
            b1c = sing.tile([128, CT], F32)
            nc.sync.dma_start(out=b1c, in_=b1_e[:, :])
            b2c = sing.tile([128, CT], F32)
            nc.sync.dma_start(out=b2c, in_=b2_e[:, :])
            dcol = sing.tile([128, CT], F32)
            nc.sync.dma_start(out=dcol, in_=d_e[:, :])

            # ---------------- kernel build -> kpad_d ----------------
            import os as _os
            _STAGES = int(_os.environ.get("KSTAGES", "4"))
            with tc.tile_pool(name="kb", bufs=2) as kb:
                zpad20 = kb.tile([128, 20], BF16)
                nc.vector.memset(zpad20, 0.0)
                offs = [0, 32, 64, 128, 256, 512]
                for ct in range(CT):
                    kdec = kb.tile([128, KLEN], F32, tag="kdec")
                    for i in range(NS):
                        rep = 2 ** max(0, i - 1)
                        kraw = kb.tile([128, KS], F32, tag="kraw")
                        nc.sync.dma_start(
                            out=kraw,
                            in_=ker_e[i, 0, ct * 128:(ct + 1) * 128, :])
                        ksrc = kraw[:, :]
                        src3 = bass.AP(tensor=ksrc.tensor, offset=ksrc.offset,
                                       ap=[ksrc.ap[0], ksrc.ap[1], [0, rep]])
                        dst = kdec[:, offs[i]:offs[i] + KS * rep].rearrange(
                            "p (t r) -> p t r", r=rep)
                        nc.scalar.activation(dst, src3, AF.Copy,
                                             scale=float(DECAY ** (NS - i - 1)))
                    ksq = kb.tile([128, KLEN], BF16, tag="ksq")
                    ssum = kb.tile([128, 1], F32, tag="ssum")
                    nc.scalar.activation(ksq, kdec, AF.Square, accum_out=ssum)
                    sd = kb.tile([128, 1], F32, tag="sdk")
                    nc.scalar.activation(sd, ssum, AF.Sqrt)
                    rn = kb.tile([128, 1], F32, tag="rnk")
                    nc.vector.reciprocal(rn, sd)
                    knb = kb.tile([128, KLEN], BF16, tag="knb")
                    nc.scalar.activation(knb, kdec, AF.Copy, scale=rn[:, 0:1])
                    nc.gpsimd.dma_start(
                        out=kpad_d[ct * 128:(ct + 1) * 128, 0:KLEN], in_=knb)
                    nc.gpsimd.dma_start(
                        out=kpad_d[ct * 128:(ct + 1) * 128, KLEN:], in_=zpad20)

            tc.strict_bb_all_engine_barrier()
            # ---------------- phase 1: LN1 + modulate -> ymod -------
            if _STAGES >= 2:
             with tc.tile_pool(name="p1", bufs=3) as p1, \
                 tc.tile_pool(name="p1p", bufs=1, space="PSUM") as p1p, \
                 tc.tile_pool(name="p1q", bufs=3, space="PSUM") as p1q:
                zpadN = p1.tile([128, N - L], BF16, tag="zpadN")
                nc.vector.memset(zpadN, 0.0)
                for b in range(BPC):
                    for ct in range(CT):
                        nc.gpsimd.dma_start(
                            out=ymod_d[b, ct * 128:(ct + 1) * 128, L:N],
                            in_=zpadN)
                for b in range(BPC):
                    for ch in range(NCH):
                        l0 = ch * LCH
                        xf = p1.tile([128, CT, LCH], F32, tag="xf")
                        for ct in range(CT):
                            dq = nc.sync if ct != 1 else nc.scalar
                            dq.dma_start(
                                out=xf[:, ct, :],
                                in_=x_e[b, ct * 128:(ct + 1) * 128, l0:l0 + LCH])
                        xb = p1.tile([128, CT, LCH], BF16, tag="xb")
                        x2 = p1.tile([128, CT, LCH], BF16, tag="x2")
                        for ct in range(CT):
                            if ct == 0:
                                nc.vector.tensor_copy(xb[:, ct, :], xf[:, ct, :])
                            else:
                                nc.scalar.activation(xb[:, ct, :], xf[:, ct, :],
                                                     AF.Copy)
                            nc.gpsimd.tensor_mul(x2[:, ct, :], xb[:, ct, :],
                                                 xb[:, ct, :])
                        s1p = p1p.tile([128, LCH], F32, tag="s1p")
                        s2p = p1p.tile([128, LCH], F32, tag="s2p")
                        for ct in range(CT):
                            MM(s1p, ones_bf, xb[:, ct, :],
                               start=(ct == 0), stop=(ct == CT - 1))
                        for ct in range(CT):
                            MM(s2p, ones_bf, x2[:, ct, :],
                               start=(ct == 0), stop=(ct == CT - 1))
                        mu = p1.tile([128, LCH], BF16, tag="mu")
                        ex2 = p1.tile([128, LCH], F32, tag="ex2")
                        nc.scalar.activation(mu, s1p, AF.Copy, scale=1.0 / C)
                        nc.scalar.activation(ex2, s2p, AF.Copy, scale=1.0 / C)
                        musq = p1.tile([128, LCH], F32, tag="musq")
                        nc.gpsimd.tensor_mul(musq, mu, mu)
                        var = p1.tile([128, LCH], F32, tag="var")
                        nc.vector.tensor_sub(var, ex2, musq)
                        sd_ = p1.tile([128, LCH], F32, tag="sd_")
                        nc.scalar.activation(sd_, var, AF.Sqrt, bias=EPS)
                        inv = p1.tile([128, LCH], BF16, tag="inv")
                        nc.vector.reciprocal(inv, sd_)
                        muinv = p1.tile([128, LCH], BF16, tag="muinv")
                        nc.vector.tensor_mul(muinv, mu, inv)
                        tcf = p1.tile([128, LCH], F32, tag="tcf")
                        nc.sync.dma_start(out=tcf, in_=tc_e[b, :, l0:l0 + LCH])
                        tsil = p1.tile([128, LCH], BF16, tag="tsil")
                        nc.scalar.activation(tsil, tcf, AF.Silu)
                        ym = p1.tile([128, CT, LCH], BF16, tag="ym")
                        for ct in range(CT):
                            adp = p1q.tile([128, 2, LCH], F32, tag="adp")
                            MM(adp[:, 0, :], adawT[:, ct, :], tsil,
                               start=True, stop=True)
                            MM(adp[:, 1, :], adawT[:, 3 + ct, :], tsil,
                               start=True, stop=True)
                            m1 = p1.tile([128, LCH], BF16, tag=f"m1_{ct}")
                            nc.gpsimd.tensor_mul(m1, xb[:, ct, :], inv)
                            z = p1.tile([128, LCH], BF16, tag=f"z_{ct}")
                            nc.gpsimd.tensor_sub(z, m1, muinv)
                            t_ = p1.tile([128, LCH], BF16, tag=f"t_{ct}")
                            nc.vector.scalar_tensor_tensor(
                                t_, adp[:, 1, :], adab[:, 3 + ct:4 + ct],
                                z, op0=AX.add, op1=AX.mult)
                            y1 = p1.tile([128, LCH], BF16, tag=f"y1_{ct}")
                            nc.gpsimd.tensor_add(y1, t_, z)
                            nc.vector.scalar_tensor_tensor(
                                ym[:, ct, :], adp[:, 0, :],
                                adab[:, ct:ct + 1], y1, op0=AX.add, op1=AX.add)
                        for ct in range(CT):
                            nc.gpsimd.dma_start(
                                out=ymod_d[b, ct * 128:(ct + 1) * 128,
                                           l0:l0 + LCH],
                                in_=ym[:, ct, :])

            tc.strict_bb_all_engine_barrier()
            # ---------------- phase 2: kernel FFT + conv FFT --------
            if _STAGES >= 3:
             with tc.tile_pool(name="p2", bufs=1) as p2, \
                 tc.tile_pool(name="p2in", bufs=2) as p2in, \
                 tc.tile_pool(name="p2p", bufs=1, space="PSUM") as p2p:

                H2 = [(0, 20), (20, 16)]    # n2 halves (start, count)
                HS = [(0, 8), (8, 8)]        # subgroup halves

                def pair(h):
                    t = "a" if h == 0 else "b"
                    pre = p2p.tile([128, 1024], F32, tag=f"p{t}_re")
                    pim = p2p.tile([128, 1024], F32, tag=f"p{t}_im")
                    return pre, pim

                def unscr_half(dst, psrc, h, eng):
                    # psrc [128,1024]: 2 banks of 10 slots x 48 -> dst ch-major
                    d3 = dst.rearrange("p (c n) -> p c n", n=N2)
                    st, cnt = H2[h]
                    s5 = psrc.rearrange("p (bk r) -> p bk r", bk=2)[
                        :, :, 0:480].rearrange("p bk (sl c) -> p bk sl c",
                                               sl=10)
                    act = eng is nc.scalar
                    if cnt == 20:
                        o = d3[:, :, st:st + 20].rearrange(
                            "p c (bk sl) -> p bk sl c", bk=2)
                        if act:
                            eng.activation(o, s5[:, :, :, 0:48], AF.Copy)
                        else:
                            eng.tensor_copy(o, s5[:, :, :, 0:48])
                    else:
                        o1 = d3[:, :, st:st + 10].rearrange("p c n -> p n c")
                        o2 = d3[:, :, st + 10:st + 16].rearrange(
                            "p c n -> p n c")
                        if act:
                            eng.activation(o1, s5[:, 0, :, 0:48], AF.Copy)
                            eng.activation(o2, s5[:, 1, 0:6, 0:48], AF.Copy)
                        else:
                            eng.tensor_copy(o1, s5[:, 0, :, 0:48])
                            eng.tensor_copy(o2, s5[:, 1, 0:6, 0:48])

                def f1_half(pre, pim, h, zr, zi, real):
                    st, cnt = H2[h]
                    for jx in range(cnt):
                        n2 = st + jx
                        off = (jx // 10) * 512 + (jx % 10) * 48
                        if real:
                            MM(pre[:, off:off + GS], sb["fA_re"][0:29, n2, :],
                               zr[:, :, n2], start=True, stop=True)
                            MM(pim[:, off:off + GS], sb["fA_im"][0:29, n2, :],
                               zr[:, :, n2], start=True, stop=True)
                        else:
                            MM(pre[:, off:off + GS], sb["fA_re"][:, n2, :],
                               zr[:, :, n2], start=True, stop=False)
                            MM(pim[:, off:off + GS], sb["fA_im"][:, n2, :],
                               zr[:, :, n2], start=True, stop=False)
                            MM(pre[:, off:off + GS], sb["fA_imn"][:, n2, :],
                               zi[:, :, n2], start=False, stop=True)
                            MM(pim[:, off:off + GS], sb["fA_re"][:, n2, :],
                               zi[:, :, n2], start=False, stop=True)

                def i4_half(pre, pim, h, vr3, vi3):
                    st, cnt = H2[h]
                    for jx in range(cnt):
                        n2 = st + jx
                        off = (jx // 10) * 512 + (jx % 10) * 48
                        MM(pre[:, off:off + GS], sb["iA_re"][:, n2, :],
                           vr3[:, :, n2], start=True, stop=False)
                        MM(pim[:, off:off + GS], sb["iA_im"][:, n2, :],
                           vr3[:, :, n2], start=True, stop=False)
                        MM(pre[:, off:off + GS], sb["iA_imn"][:, n2, :],
                           vi3[:, :, n2], start=False, stop=True)
                        MM(pim[:, off:off + GS], sb["iA_re"][:, n2, :],
                           vi3[:, :, n2], start=False, stop=True)

                def t_half(pre, pim, h, inre, inim):
                    st, _ = HS[h]
                    for s in range(st, st + 8):
                        off = ((s - st) // 4) * 512 + ((s - st) % 4) * 128
                        isl = slice(s * 108, (s + 1) * 108)
                        MM(pre[:108, off:off + 128], inre[:, isl], ident32,
                           is_transpose=True, start=True, stop=True)
                        MM(pim[:108, off:off + 128], inim[:, isl], ident32,
                           is_transpose=True, start=True, stop=True)

                def tb_half(pre, pim, h, inre, inim):
                    st, _ = HS[h]
                    for s in range(st, st + 8):
                        off = ((s - st) // 4) * 512 + ((s - st) % 4) * 108
                        isl = slice(s * 128, (s + 1) * 128)
                        MM(pre[:, off:off + 108], inre[:108, isl],
                           ident32[:108, :108], is_transpose=True, start=True,
                           stop=True)
                        MM(pim[:, off:off + 108], inim[:108, isl],
                           ident32[:108, :108], is_transpose=True, start=True,
                           stop=True)

                def d36_half(pre, pim, h, Bre, Bim, Bimn, inre, inim):
                    st, _ = HS[h]
                    for s in range(st, st + 8):
                        off = (s - st) * 128
                        sl = slice(s * 128, (s + 1) * 128)
                        MM(pre[:108, off:off + 128], Bre, inre[:, sl],
                           start=True, stop=False)
                        MM(pim[:108, off:off + 128], Bim, inre[:, sl],
                           start=True, stop=False)
                        MM(pre[:108, off:off + 128], Bimn, inim[:, sl],
                           start=False, stop=True)
                        MM(pim[:108, off:off + 128], Bre, inim[:, sl],
                           start=False, stop=True)

                def hcopy(dst, psrc, h, eng):
                    o = dst[:, h * 1024:(h + 1) * 1024]
                    if eng is nc.scalar:
                        eng.activation(o, psrc[:108, :], AF.Copy)
                    else:
                        eng.tensor_copy(o, psrc[:108, :])

                def vcopy_half(vflat, psrc, h, eng):
                    for bk in range(2):
                        o = vflat[:, (h * 8 + bk * 4) * 108:
                                  (h * 8 + bk * 4) * 108 + 432]
                        s_ = psrc[:, bk * 512:bk * 512 + 432]
                        if eng is nc.scalar:
                            eng.activation(o, s_, AF.Copy)
                        else:
                            eng.tensor_copy(o, s_)

                for g in range(NG):
                    c0 = g * GS
                    # ======== kernel FFT for this group ========
                    kz = p2in.tile([29, GS, N2], BF16, tag="kz")
                    nc.sync.dma_start(
                        out=kz, in_=kpad_d[c0:c0 + GS, :].rearrange(
                            "c (a b) -> a c b", b=N2))
                    S_re = p2.tile([128, GS * N2], F32, tag="S_re")
                    S_im = p2.tile([128, GS * N2], F32, tag="S_im")
                    for h in (0, 1):
                        pre, pim = pair(h)
                        f1_half(pre, pim, h, kz, None, True)
                        unscr_half(S_re, pre, h, nc.scalar)
                        unscr_half(S_im, pim, h, nc.vector)
                    ST_re = p2.tile([108, NSUB * 128], BF16, tag="ST_re")
                    ST_im = p2.tile([108, NSUB * 128], BF16, tag="ST_im")
                    for h in (0, 1):
                        pre, pim = pair(h)
                        t_half(pre, pim, h, S_re, S_im)
                        hcopy(ST_re, pre, h, nc.scalar)
                        hcopy(ST_im, pim, h, nc.vector)
                    kh_re = p2.tile([108, NSUB * 128], BF16, tag="kh_re")
                    kh_im = p2.tile([108, NSUB * 128], BF16, tag="kh_im")
                    for h in (0, 1):
                        pre, pim = pair(h)
                        d36_half(pre, pim, h, sb["fB_re"], sb["fB_im"],
                                 sb["fB_imn"], ST_re, ST_im)
                        hcopy(kh_re, pre, h, nc.scalar)
                        hcopy(kh_im, pim, h, nc.vector)

                    # ======== data FFT (batch pair packed complex) ========
                    z_re = p2in.tile([128, GS, N2], BF16, tag="z_re")
                    z_im = p2in.tile([128, GS, N2], BF16, tag="z_im")
                    nc.sync.dma_start(
                        out=z_re, in_=ymod_d[0, c0:c0 + GS, :].rearrange(
                            "c (a b) -> a c b", b=N2))
                    nc.sync.dma_start(
                        out=z_im, in_=ymod_d[1, c0:c0 + GS, :].rearrange(
                            "c (a b) -> a c b", b=N2))
                    S_re = p2.tile([128, GS * N2], F32, tag="S_re")
                    S_im = p2.tile([128, GS * N2], F32, tag="S_im")
                    for h in (0, 1):
                        pre, pim = pair(h)
                        f1_half(pre, pim, h, z_re, z_im, False)
                        unscr_half(S_re, pre, h, nc.scalar)
                        unscr_half(S_im, pim, h, nc.vector)
                    ST_re = p2.tile([108, NSUB * 128], BF16, tag="ST_re")
                    ST_im = p2.tile([108, NSUB * 128], BF16, tag="ST_im")
                    for h in (0, 1):
                        pre, pim = pair(h)
                        t_half(pre, pim, h, S_re, S_im)
                        hcopy(ST_re, pre, h, nc.scalar)
                        hcopy(ST_im, pim, h, nc.vector)
                    X_re = p2.tile([108, NSUB * 128], BF16, tag="X_re")
                    X_im = p2.tile([108, NSUB * 128], BF16, tag="X_im")
                    for h in (0, 1):
                        pre, pim = pair(h)
                        d36_half(pre, pim, h, sb["fB_re"], sb["fB_im"],
                                 sb["fB_imn"], ST_re, ST_im)
                        hcopy(X_re, pre, h, nc.scalar)
                        hcopy(X_im, pim, h, nc.vector)
                    # spectral multiply (per half to keep pipeline fine-grained)
                    Y_re = p2.tile([108, NSUB * 128], BF16, tag="Y_re")
                    Y_im = p2.tile([108, NSUB * 128], BF16, tag="Y_im")
                    q1 = p2.tile([108, NSUB * 128], BF16, tag="q1")
                    q2 = p2.tile([108, NSUB * 128], BF16, tag="q2")
                    for h in (0, 1):
                        sl = slice(h * 1024, (h + 1) * 1024)
                        nc.vector.tensor_mul(q1[:, sl], X_re[:, sl],
                                             kh_re[:, sl])
                        nc.gpsimd.tensor_mul(q2[:, sl], X_im[:, sl],
                                             kh_im[:, sl])
                        nc.gpsimd.tensor_sub(Y_re[:, sl], q1[:, sl],
                                             q2[:, sl])
                        nc.vector.tensor_mul(q1[:, sl], X_re[:, sl],
                                             kh_im[:, sl])
                        nc.gpsimd.tensor_mul(q2[:, sl], X_im[:, sl],
                                             kh_re[:, sl])
                        nc.vector.tensor_add(Y_im[:, sl], q1[:, sl],
                                             q2[:, sl])
                    U_re = p2.tile([108, NSUB * 128], F32, tag="U_re")
                    U_im = p2.tile([108, NSUB * 128], F32, tag="U_im")
                    for h in (0, 1):
                        pre, pim = pair(h)
                        d36_half(pre, pim, h, sb["iB_re"], sb["iB_im"],
                                 sb["iB_imn"], Y_re, Y_im)
                        hcopy(U_re, pre, h, nc.scalar)
                        hcopy(U_im, pim, h, nc.vector)
                    V_re = p2.tile([128, GS, N2], BF16, tag="V_re")
                    V_im = p2.tile([128, GS, N2], BF16, tag="V_im")
                    vr = V_re.rearrange("p a b -> p (a b)")
                    vi = V_im.rearrange("p a b -> p (a b)")
                    for h in (0, 1):
                        pre, pim = pair(h)
                        tb_half(pre, pim, h, U_re, U_im)
                        vcopy_half(vr, pre, h, nc.scalar)
                        vcopy_half(vi, pim, h, nc.vector)
                    yo_re = p2.tile([128, GS, N2], BF16, tag="yo_re")
                    yo_im = p2.tile([128, GS, N2], BF16, tag="yo_im")
                    yof_re = yo_re.rearrange("p a b -> p (a b)")
                    yof_im = yo_im.rearrange("p a b -> p (a b)")
                    for h in (0, 1):
                        pre, pim = pair(h)
                        i4_half(pre, pim, h, V_re, V_im)
                        unscr_half(yof_re, pre, h, nc.scalar)
                        unscr_half(yof_im, pim, h, nc.vector)
                    nc.gpsimd.dma_start(
                        out=yconv_d[0, c0:c0 + GS, :].rearrange(
                            "c (a b) -> a c b", b=N2), in_=yo_re)
                    nc.gpsimd.dma_start(
                        out=yconv_d[1, c0:c0 + GS, :].rearrange(
                            "c (a b) -> a c b", b=N2), in_=yo_im)

            tc.strict_bb_all_engine_barrier()
            # ---------------- phase 3 ------------------------------
            if _STAGES >= 4:
             with tc.tile_pool(name="p3", bufs=1) as p3, \
                 tc.tile_pool(name="p3p", bufs=1, space="PSUM") as p3p, \
                 tc.tile_pool(name="p3pm", bufs=2, space="PSUM") as p3pm:
                for ch in range(NCH):
                    for b in range(BPC):
                        l0 = ch * LCH
                        yc = p3.tile([128, CT, LCH], BF16, tag="yc3" + str(b))
                        ym3 = p3.tile([128, CT, LCH], BF16, tag="ym3" + str(b))
                        xb = p3.tile([128, CT, LCH], BF16, tag="xb3" + str(b))
                        for ct in range(CT):
                            xstg = p3.tile([128, LCH], F32, tag="xstg" + str(b))
                            nc.sync.dma_start(
                                out=xstg,
                                in_=x_e[b, ct * 128:(ct + 1) * 128, l0:l0 + LCH])
                            nc.sync.dma_start(
                                out=yc[:, ct, :],
                                in_=yconv_d[b, ct * 128:(ct + 1) * 128,
                                            SS + l0:SS + l0 + LCH])
                            nc.sync.dma_start(
                                out=ym3[:, ct, :],
                                in_=ymod_d[b, ct * 128:(ct + 1) * 128,
                                           l0:l0 + LCH])
                            nc.scalar.activation(xb[:, ct, :], xstg, AF.Copy)
                        tcf = p3.tile([128, LCH], F32, tag="tcf3" + str(b))
                        nc.sync.dma_start(out=tcf, in_=tc_e[b, :, l0:l0 + LCH])
                        tsil = p3.tile([128, LCH], BF16, tag="tsil3" + str(b))
                        nc.scalar.activation(tsil, tcf, AF.Silu)
                        # gate_tm chunks -> SBUF
                        gts = p3.tile([128, CT, LCH], BF16, tag="gts" + str(b))
                        for ct in range(CT):
                            adp3 = p3pm.tile([128, LCH], F32, tag="adp3")
                            MM(adp3, adawT[:, 6 + ct, :], tsil,
                               start=True, stop=True)
                            nc.vector.tensor_scalar(
                                gts[:, ct, :], adp3, adab[:, 6 + ct:7 + ct],
                                None, AX.add)
                        x1 = p3.tile([128, CT, LCH], BF16, tag="x1" + str(b))
                        x2t = p3.tile([128, CT, LCH], BF16, tag="x2t" + str(b))
                        for ct in range(CT):
                            s1 = p3.tile([128, LCH], BF16, tag=f"s1_{ct}_{b}")
                            nc.vector.scalar_tensor_tensor(
                                s1, ym3[:, ct, :], dcol[:, ct:ct + 1],
                                yc[:, ct, :], op0=AX.mult, op1=AX.add)
                            gt = p3.tile([128, LCH], BF16, tag=f"gt_{ct}_{b}")
                            nc.vector.tensor_mul(gt, gts[:, ct, :], s1)
                            nc.gpsimd.tensor_add(x1[:, ct, :], xb[:, ct, :], gt)
                            nc.gpsimd.tensor_mul(x2t[:, ct, :], x1[:, ct, :],
                                                 x1[:, ct, :])
                        s1p3 = p3p.tile([128, LCH], F32, tag="s1p3" + str(b))
                        s2p3 = p3p.tile([128, LCH], F32, tag="s2p3" + str(b))
                        for ct in range(CT):
                            MM(s1p3, ones_bf, x1[:, ct, :],
                               start=(ct == 0), stop=(ct == CT - 1))
                        for ct in range(CT):
                            MM(s2p3, ones_bf, x2t[:, ct, :],
                               start=(ct == 0), stop=(ct == CT - 1))
                        mu = p3.tile([128, LCH], BF16, tag="mu3" + str(b))
                        ex2 = p3.tile([128, LCH], BF16, tag="ex23" + str(b))
                        nc.scalar.activation(mu, s1p3, AF.Copy,
                                             scale=1.0 / C)
                        nc.scalar.activation(ex2, s2p3, AF.Copy,
                                             scale=1.0 / C)
                        musq = p3.tile([128, LCH], BF16, tag="musq3" + str(b))
                        nc.gpsimd.tensor_mul(musq, mu, mu)
                        var = p3.tile([128, LCH], BF16, tag="var3" + str(b))
                        nc.vector.tensor_sub(var, ex2, musq)
                        sd_ = p3.tile([128, LCH], BF16, tag="sd_3" + str(b))
                        nc.scalar.activation(sd_, var, AF.Sqrt, bias=EPS)
                        inv = p3.tile([128, LCH], BF16, tag="inv3" + str(b))
                        nc.vector.reciprocal(inv, sd_)
                        muinv = p3.tile([128, LCH], BF16, tag="muinv3" + str(b))
                        nc.vector.tensor_mul(muinv, mu, inv)
                        # ada chunks 9-14 -> SBUF
                        cms = p3.tile([128, 6, LCH], BF16, tag="cms" + str(b))
                        for oc in range(6):
                            adp3 = p3pm.tile([128, LCH], F32, tag="adp3")
                            MM(adp3, adawT[:, 9 + oc, :], tsil,
                               start=True, stop=True)
                            nc.scalar.activation(cms[:, oc, :], adp3, AF.Identity,
                                                 bias=adab[:, 9 + oc:10 + oc])
                        z2 = p3.tile([128, CT, LCH], BF16, tag="z2" + str(b))
                        for ct in range(CT):
                            m1 = p3.tile([128, LCH], BF16, tag=f"m13_{ct}_{b}")
                            nc.gpsimd.tensor_mul(m1, x1[:, ct, :], inv)
                            z = p3.tile([128, LCH], BF16, tag=f"z3_{ct}_{b}")
                            nc.gpsimd.tensor_sub(z, m1, muinv)
                            t_ = p3.tile([128, LCH], BF16, tag=f"t3_{ct}_{b}")
                            nc.vector.tensor_mul(t_, cms[:, 3 + ct, :], z)
                            y1 = p3.tile([128, LCH], BF16, tag=f"y13_{ct}_{b}")
                            nc.gpsimd.tensor_add(y1, t_, z)
                            nc.vector.tensor_add(z2[:, ct, :],
                                                 cms[:, ct, :], y1)
                        # MLP layer 1
                        h = p3.tile([128, CT, LCH], BF16, tag="h" + str(b))
                        for oc in range(CT):
                            hp = p3pm.tile([128, LCH], F32, tag="mlp")
                            for ct in range(CT):
                                MM(hp, w1T[:, ct, oc * 128:(oc + 1) * 128],
                                   z2[:, ct, :], start=(ct == 0),
                                   stop=(ct == CT - 1))
                            nc.scalar.activation(h[:, oc, :], hp, AF.Gelu,
                                                 bias=b1c[:, oc:oc + 1])
                        # gate_cm -> SBUF
                        gcs = p3.tile([128, CT, LCH], BF16, tag="gcs" + str(b))
                        for ct in range(CT):
                            adp3 = p3pm.tile([128, LCH], F32, tag="adp3")
                            MM(adp3, adawT[:, 15 + ct, :], tsil,
                               start=True, stop=True)
                            nc.vector.tensor_scalar(
                                gcs[:, ct, :], adp3, adab[:, 15 + ct:16 + ct],
                                None, AX.add)
                        for oc in range(CT):
                            mp = p3pm.tile([128, LCH], F32, tag="mlp")
                            for ct in range(CT):
                                MM(mp, w2T[:, ct, oc * 128:(oc + 1) * 128],
                                   h[:, ct, :], start=(ct == 0),
                                   stop=(ct == CT - 1))
                            mb = p3.tile([128, LCH], BF16, tag=f"mb_{oc}_{b}")
                            nc.scalar.activation(mb, mp, AF.Identity,
                                                 bias=b2c[:, oc:oc + 1])
                            gc = p3.tile([128, LCH], BF16, tag=f"gc_{oc}_{b}")
                            nc.vector.tensor_mul(gc, gcs[:, oc, :], mb)
                            ostg = p3.tile([128, LCH], F32, tag="ostg" + str(b))
                            nc.vector.tensor_add(ostg, x1[:, oc, :], gc)
                            nc.gpsimd.dma_start(
                                out=out_e[b, oc * 128:(oc + 1) * 128,
                                          l0:l0 + LCH],
                                in_=ostg)
    nc.finalize()
    return nc


def kernel(x, t_cond, kernels, D, ada_w, ada_b, w1, b1, w2, b2):
    global _last_results
    consts = _make_consts()
    nc = build_graph()
    shared = {
        "kernels": np.ascontiguousarray(kernels, dtype=np.float32),
        "DT": np.ascontiguousarray(
            np.asarray(D, np.float32).reshape(CT, 128).T),
        "ada_wT": np.ascontiguousarray(ada_w.T).astype(ml_dtypes.bfloat16),
        "ada_bT": np.ascontiguousarray(
            np.asarray(ada_b, np.float32).reshape(18, 128).T),
        "w1T": np.ascontiguousarray(w1.T).astype(ml_dtypes.bfloat16),
        "b1T": np.ascontiguousarray(
            np.asarray(b1, np.float32).reshape(CT, 128).T),
        "w2T": np.ascontiguousarray(w2.T).astype(ml_dtypes.bfloat16),
        "b2T": np.ascontiguousarray(
            np.asarray(b2, np.float32).reshape(CT, 128).T),
    }
    shared.update(consts)
    in_maps = []
    for i in range(NCORES):
        m = dict(shared)
        m["x"] = np.ascontiguousarray(x[i * BPC:(i + 1) * BPC], dtype=np.float32)
        m["t_cond"] = np.ascontiguousarray(t_cond[i * BPC:(i + 1) * BPC],
                                           dtype=np.float32)
        in_maps.append(m)
    trace = os.environ.get("KERNEL_TRACE", "0") == "1"
    res = run_bass_kernel_spmd(nc, in_maps, list(range(NCORES)), trace=trace)
    _last_results = res
    outs = [r["out"] if isinstance(r, dict) else r for r in res.results]
    return np.concatenate([np.asarray(o, dtype=np.float32).reshape(BPC, C, L)
                           for o in outs], axis=0)


if __name__ == "__main__":
    build_graph()
    print("graph built ok")

